# revision 22
# baseline (speedup 1.0000x reference)
"""GRU decoder Trainium2 kernel (data-parallel over batch, 8 cores).

Reference (per step t, PyTorch nn.GRU gate order r,z,n):
    gi = x @ w_ih.T + b_ih ; gh = h @ w_hh.T + b_hh
    r = sig(i_r + h_r); z = sig(i_z + h_z); n = tanh(i_n + r * h_n)
    h' = (1-z)*n + z*h ; y = h' @ w_fc.T + b_fc ; x <- y
Shapes: H=1024, O=768, B=256, T=256.  Each core handles 32 batch rows.

Key restructuring vs the straightforward mapping:
  * Since x_t = y_{t-1} = h'_{t-1} @ w_fc.T + b_fc, the input-side GRU
    matmuls fold into the hidden-side ones:
        r/z gates:  h' @ (w_fc.T @ w_ih_g.T + w_hh_g.T) + b_eff_g
        i_n:        h' @ (w_fc.T @ w_ih_n.T) + b_eff_in
        h_n:        h' @ w_hh_n.T + b_hh_n
    so every recurrent matmul contracts over H=1024 (not O+H=1792), and
    y_t itself is only needed for the OUTPUT -> off the critical path.
  * Step 0 gates are computed on the host from (src[0], hidden[0]).
  * Per core, batch=32 lives in the PE stationary free dim; 4 column
    groups (tile_position=(0,32j)) cover feature quadrants concurrently.
  * Gate regions are issued region-major in order r, h_n, i_n, z so the
    sigmoid/tanh chain for region g overlaps the matmuls of regions > g.
  * Per-step gate biases are seeded into PSUM with K=1 matmuls
    (ones[1,32] x bias_row[1,256]) as the start=True instruction of each
    accumulation group - no bias adds on the vector chain.
  * b_fc is added on the host at the end.
"""

import numpy as np
import ml_dtypes

import concourse.bass as bass
import concourse.bacc as bacc
import concourse.tile as tile
from concourse import mybir
from concourse.bass_utils import run_bass_kernel_spmd

H = 1024
O = 768
B = 256
T = 256
NCORES = 8
BC = B // NCORES  # 32 batch rows per core

KH = H // 128  # 8 contraction chunks
NGATE = 4      # regions r, hn, in, z (issue order)
YW = O // 4    # 192 y cols per quadrant

F32 = mybir.dt.float32
BF16 = mybir.dt.bfloat16
AF = mybir.ActivationFunctionType
ALU = mybir.AluOpType

_COMPILED = None

# bf16 const layout: WG | WF | ONES | BIAS
WG_N = NGATE * KH * 4 * 256   # 32768
WF_N = KH * 4 * YW            # 6144
NB = WG_N + WF_N + 32 + 4096  # 43040
# f32 const layout: G0 | HP0 | IDT | BV (packed per-quadrant bias tiles)
NF = NGATE * 256 + 256 + 128 + NGATE * 256  # 2432

# 'matmul': seed biases into PSUM with K=1 ones-row matmuls
# 'vector': add biases on the vector engine during the chain
BIAS_MODE = "matmul"


def _hslice(hsb0, hsb1, k):
    """lhsT chunk k (h features 128k..128k+128) from packed h'^T half-tiles."""
    c = 32 * (k // 2)
    src = hsb0 if k % 2 == 0 else hsb1
    return src[:, c : c + 32]


def _build_nc():
    nc = bacc.Bacc("TRN2", target_bir_lowering=False, debug=False, num_devices=NCORES)

    cb = nc.declare_dram_parameter("CB", [128, NB], BF16, isOutput=False)
    cf = nc.declare_dram_parameter("CF", [128, NF], F32, isOutput=False)
    o = nc.declare_dram_parameter("O", [T, 128, YW], F32, isOutput=True)

    with tile.TileContext(nc) as tc:
        with (
            tc.tile_pool(name="wpool", bufs=1) as wpool,
            tc.tile_pool(name="state", bufs=2) as spool,
            tc.tile_pool(name="act", bufs=2) as apool,
            tc.tile_pool(name="gps", bufs=1, space="PSUM") as gpool,
            tc.tile_pool(name="tps", bufs=2, space="PSUM") as tpool,
            tc.tile_pool(name="yps", bufs=2, space="PSUM") as ypool,
        ):
            CB = wpool.tile([128, NB], BF16, tag="CB")
            CF = wpool.tile([128, NF], F32, tag="CF")
            nc.sync.dma_start(CB[:], cb[:])
            nc.sync.dma_start(CF[:], cf[:])
            WG = CB[:, 0:WG_N]
            WF = CB[:, WG_N : WG_N + WF_N]
            ONES = CB[0:1, WG_N + WF_N : WG_N + WF_N + 32]
            BIAS = CB[0:1, WG_N + WF_N + 32 : NB]
            G0 = CF[:, 0 : NGATE * 256]
            HPc = CF[:, NGATE * 256 : NGATE * 256 + 256]
            IDT = CF[:, NGATE * 256 + 256 : NGATE * 256 + 384]
            BV = CF[:, NGATE * 256 + 384 : NF]  # r|hn|in|z packed [128,256] each

            HpL = spool.tile([128, 128], F32, tag="HpL")
            HpH = spool.tile([128, 128], F32, tag="HpH")
            nc.vector.tensor_copy(HpL[:], HPc[:, 0:128])
            nc.vector.tensor_copy(HpH[:], HPc[:, 128:256])

            def chain(r_src, hn_src, in_src, z_src, hpL, hpH):
                """gates -> h'.  Full-width rs/rt/ns, then a half-split tail
                (tanh/zs on scalar, d/e/hp2 on vector) in separate tiles so
                scalar+vector pipeline and the next transpose starts per-half.
                """
                rs = apool.tile([128, 256], F32, tag="rs")
                nc.scalar.activation(rs[:], r_src, AF.Sigmoid)
                rt = apool.tile([128, 256], F32, tag="rt")
                nc.vector.tensor_tensor(rt[:], rs[:], hn_src, ALU.mult)
                ns = apool.tile([128, 256], F32, tag="ns")
                nc.vector.tensor_tensor(ns[:], rt[:], in_src, ALU.add)
                nL = apool.tile([128, 128], F32, tag="nL")
                nH = apool.tile([128, 128], F32, tag="nH")
                nc.scalar.activation(nL[:], ns[:, 0:128], AF.Tanh)
                nc.scalar.activation(nH[:], ns[:, 128:256], AF.Tanh)
                zL = apool.tile([128, 128], F32, tag="zL")
                zH = apool.tile([128, 128], F32, tag="zH")
                nc.scalar.activation(zL[:], z_src[:, 0:128], AF.Sigmoid)
                nc.scalar.activation(zH[:], z_src[:, 128:256], AF.Sigmoid)
                dL = apool.tile([128, 128], F32, tag="dL")
                dH = apool.tile([128, 128], F32, tag="dH")
                nc.vector.tensor_tensor(dL[:], hpL[:], nL[:], ALU.subtract)
                nc.vector.tensor_tensor(dH[:], hpH[:], nH[:], ALU.subtract)
                eL = apool.tile([128, 128], F32, tag="eL")
                nc.vector.tensor_tensor(eL[:], zL[:], dL[:], ALU.mult)
                hp2L = spool.tile([128, 128], F32, tag="HpL")
                nc.vector.tensor_tensor(hp2L[:], nL[:], eL[:], ALU.add)
                eH = apool.tile([128, 128], F32, tag="eH")
                nc.vector.tensor_tensor(eH[:], zH[:], dH[:], ALU.mult)
                hp2H = spool.tile([128, 128], F32, tag="HpH")
                nc.vector.tensor_tensor(hp2H[:], nH[:], eH[:], ALU.add)
                return hp2L, hp2H

            # step 0: gates computed host-side (biases already included)
            HpL, HpH = chain(
                G0[:, 0:256], G0[:, 256:512], G0[:, 512:768], G0[:, 768:1024],
                HpL, HpH,
            )

            # even chunks first: their lhsT half (hsb0) is cast first, so
            # even-k matmuls start while the odd half is still casting
            K_ORDER = [0, 2, 4, 6, 1, 3, 5, 7]

            for t in range(T):
                # ---- h'_t^T for this step's y and next step's gates ----
                # Half-tiles: deps are tile-granular, so separate tiles per
                # half let even-k matmuls start after the first cast.
                tp = tpool.tile([128, 256], F32, tag="tp")
                nc.tensor.transpose(tp[:, 0:128], HpL[:], IDT)
                nc.tensor.transpose(tp[:, 128:256], HpH[:], IDT)
                hsb0 = spool.tile([128, 128], BF16, tag="hsb0")
                hsb1 = spool.tile([128, 128], BF16, tag="hsb1")
                nc.vector.tensor_copy(hsb0[:], tp[:, 0:128])
                nc.scalar.activation(hsb1[:], tp[:, 128:256], AF.Copy)

                last = t == T - 1
                if not last:
                    # One PSUM bank per region (bufs=1: the chain drains
                    # within the step, so no double buffering needed) ->
                    # per-region dep granularity for early chain start.
                    gsl = [
                        gpool.tile([128, 256], F32, tag=f"g{g}", name=f"g{g}")
                        for g in range(NGATE)
                    ]
                    # bias seed (start=True); each region has its own bank
                    # so starts never clear another region's bias
                    for g in range(NGATE):
                        for j in range(4):
                            bofs = 1024 * g + 256 * j
                            nc.tensor.matmul(
                                gsl[g][32 * j : 32 * j + 32, :],
                                ONES[:, 0:32],
                                BIAS[:, bofs : bofs + 256],
                                start=True,
                                stop=False,
                                tile_position=(0, 32 * j),
                            )
                    # region-major gate matmuls: r, hn, in, z
                    for g in range(NGATE):
                        for k in K_ORDER:
                            lhsT = _hslice(hsb0, hsb1, k)
                            for j in range(4):
                                wofs = ((g * KH + k) * 4 + j) * 256
                                nc.tensor.matmul(
                                    gsl[g][32 * j : 32 * j + 32, :],
                                    lhsT,
                                    WG[:, wofs : wofs + 256],
                                    start=False,
                                    stop=(k == 7),
                                    tile_position=(0, 32 * j),
                                )
                    gsl = [g[:] for g in gsl]

                # ---- y_t = h'_t @ w_fc.T (output only; off critical path) ----
                yp = ypool.tile([128, YW], F32, tag="yp")
                for ki, k in enumerate(K_ORDER):
                    lhsT = _hslice(hsb0, hsb1, k)
                    for j in range(4):
                        wofs = (k * 4 + j) * YW
                        nc.tensor.matmul(
                            yp[32 * j : 32 * j + 32, :],
                            lhsT,
                            WF[:, wofs : wofs + YW],
                            start=(ki == 0),
                            stop=(ki == KH - 1),
                            tile_position=(0, 32 * j),
                        )

                if not last:
                    HpL, HpH = chain(gsl[0], gsl[1], gsl[2], gsl[3], HpL, HpH)

                ys = apool.tile([128, YW], F32, tag="ys")
                nc.vector.tensor_copy(ys[:], yp[:])
                nc.sync.dma_start(o[t], ys[:])

    nc.compile()
    return nc


def _pack_bat(M):
    """[32, 4*W] -> [128, W]: row 32j+b holds M[b, W*j : W*j+W]."""
    w = M.shape[1] // 4
    return np.ascontiguousarray(
        M.reshape(BC, 4, w).transpose(1, 0, 2).reshape(128, w)
    )


def _prep_shared(w_ih, w_hh, b_ih, b_hh, w_fc, b_fc):
    wihT = w_ih.T.astype(np.float64)  # [768, 3072]
    whhT = w_hh.T.astype(np.float64)  # [1024, 3072]
    wfcT = w_fc.T.astype(np.float64)  # [1024, 768]
    fold = wfcT @ wihT                # [1024, 3072]
    Wr = fold[:, 0:H] + whhT[:, 0:H]
    Wz = fold[:, H : 2 * H] + whhT[:, H : 2 * H]
    Win = fold[:, 2 * H : 3 * H]
    Whn = whhT[:, 2 * H : 3 * H]

    bfold = b_fc.astype(np.float64) @ wihT  # [3072]
    br = bfold[0:H] + b_ih[0:H] + b_hh[0:H]
    bz = bfold[H : 2 * H] + b_ih[H : 2 * H] + b_hh[H : 2 * H]
    bin_ = bfold[2 * H :] + b_ih[2 * H :]
    bhn = b_hh[2 * H :].astype(np.float64)

    blocks = []
    for G in (Wr, Whn, Win, Wz):  # region order r, hn, in, z
        for k in range(KH):
            for j in range(4):
                blocks.append(G[128 * k : 128 * k + 128, 256 * j : 256 * j + 256])
    WGp = np.concatenate(blocks, axis=1).astype(ml_dtypes.bfloat16)  # [128, 32768]

    yblocks = []
    for k in range(KH):
        for j in range(4):
            yblocks.append(wfcT[128 * k : 128 * k + 128, YW * j : YW * j + YW])
    WFp = np.concatenate(yblocks, axis=1).astype(ml_dtypes.bfloat16)  # [128, 6144]

    ones_col = np.zeros((128, 32), ml_dtypes.bfloat16)
    ones_col[0, :] = 1
    # bias layout: 1024*g + 256*j (regions r, hn, in, z)
    bias_col = np.zeros((128, 4096), ml_dtypes.bfloat16)
    bias_col[0, :] = np.concatenate([br, bhn, bin_, bz]).astype(ml_dtypes.bfloat16)

    CBp = np.concatenate([WGp, WFp, ones_col, bias_col], axis=1)  # [128, NB]
    assert CBp.shape[1] == NB
    IDT = np.eye(128, dtype=np.float32)

    def pack_bias(vec):  # [1024] -> [128, 256]: row 32j+b holds vec[256j:256j+256]
        return np.repeat(vec.reshape(4, 256), BC, axis=0).astype(np.float32)

    BVp = np.concatenate(
        [pack_bias(v) for v in (br, bhn, bin_, bz)], axis=1
    )  # [128, 1024] f32
    return CBp, IDT, BVp


def _build_in_maps(inputs):
    src = np.asarray(inputs["src"], np.float32)
    hidden = np.asarray(inputs["hidden"], np.float32)
    w_ih = np.asarray(inputs["w_ih"], np.float32)
    w_hh = np.asarray(inputs["w_hh"], np.float32)
    b_ih = np.asarray(inputs["b_ih"], np.float32)
    b_hh = np.asarray(inputs["b_hh"], np.float32)
    w_fc = np.asarray(inputs["w_fc"], np.float32)
    b_fc = np.asarray(inputs["b_fc"], np.float32)

    CBp, IDT, BVp = _prep_shared(w_ih, w_hh, b_ih, b_hh, w_fc, b_fc)

    # step-0 gates on host (f64): from x0=src[0], h0=hidden[0]
    x0 = src[0].astype(np.float64)   # [256, 768]
    h0 = hidden[0].astype(np.float64)  # [256, 1024]
    gi0 = x0 @ w_ih.T.astype(np.float64) + b_ih.astype(np.float64)
    gh0 = h0 @ w_hh.T.astype(np.float64) + b_hh.astype(np.float64)
    g0r = gi0[:, 0:H] + gh0[:, 0:H]
    g0z = gi0[:, H : 2 * H] + gh0[:, H : 2 * H]
    g0in = gi0[:, 2 * H :]
    g0hn = gh0[:, 2 * H :]

    in_maps = []
    for c in range(NCORES):
        sl = slice(BC * c, BC * (c + 1))
        G0 = np.concatenate(
            [
                _pack_bat(g0r[sl]),
                _pack_bat(g0hn[sl]),
                _pack_bat(g0in[sl]),
                _pack_bat(g0z[sl]),
            ],
            axis=1,
        )  # [128, 1024]
        HP0 = _pack_bat(h0[sl])  # [128, 256]
        CFp = np.concatenate([G0, HP0, IDT, BVp], axis=1).astype(np.float32)
        assert CFp.shape[1] == NF
        in_maps.append(dict(CB=CBp, CF=CFp))
    return in_maps


def kernel(src, tgt, hidden, w_ih, w_hh, b_ih, b_hh, w_fc, b_fc, **_kw):
    global _COMPILED
    b_fc = np.asarray(b_fc, np.float32)

    if _COMPILED is None:
        _COMPILED = _build_nc()
    nc = _COMPILED

    in_maps = _build_in_maps(
        dict(src=src, hidden=hidden, w_ih=w_ih, w_hh=w_hh, b_ih=b_ih,
             b_hh=b_hh, w_fc=w_fc, b_fc=b_fc)
    )

    res = run_bass_kernel_spmd(nc, in_maps, list(range(NCORES)))

    out = np.empty((T, B, O), np.float32)
    for c in range(NCORES):
        sl = slice(BC * c, BC * (c + 1))
        oc = np.asarray(res.results[c]["O"])  # [T, 128, 192]
        out[:, sl, :] = (
            oc.reshape(T, 4, BC, YW).transpose(0, 2, 1, 3).reshape(T, BC, O)
        )
    out += b_fc[None, None, :]
    return out


# revision 27
# speedup vs baseline: 1.1574x; 1.1574x over previous
"""GRU decoder Trainium2 kernel (data-parallel over batch, 8 cores).

Reference (per step t, PyTorch nn.GRU gate order r,z,n):
    gi = x @ w_ih.T + b_ih ; gh = h @ w_hh.T + b_hh
    r = sig(i_r + h_r); z = sig(i_z + h_z); n = tanh(i_n + r * h_n)
    h' = (1-z)*n + z*h ; y = h' @ w_fc.T + b_fc ; x <- y
Shapes: H=1024, O=768, B=256, T=256.  Each core handles 32 batch rows.

Structure (v6 - transposed state):
  * x_t = y_{t-1} folds into the hidden-side matmuls, so every recurrent
    matmul contracts over H=1024: regions r, hn (= h_n), z, in (= i_n).
  * The state lives ONLY as hsb = h'^T (bf16, PE lhsT layout).  The
    chain computes zs/n in normal layout, transposes zs and n (PE,
    cheap, off the critical tail), then finishes in transposed space:
        hsb' = n^T (1 - z^T) + z^T hsb
    so NOTHING follows the last vector op before the next gate matmuls.
  * Gate PSUM: pair tile [r|hn] (one bias MM of N=512 per quadrant -
    a second start=True in the same bank strip clears has_written and
    loses the earlier bias) + separate z / in banks, all double-buffered.
  * Biases seed PSUM via K=1 ones-row matmuls during the chain window.
  * Step-0 gates come from the host; b_fc is added on the host.
"""

import numpy as np
import ml_dtypes

import concourse.bass as bass
import concourse.bacc as bacc
import concourse.tile as tile
from concourse import mybir
from concourse.bass_utils import run_bass_kernel_spmd

H = 1024
O = 768
B = 256
T = 256
NCORES = 8
BC = B // NCORES  # 32 batch rows per core

KH = H // 128  # 8 contraction chunks
NGATE = 4      # regions r, hn, z, in (issue order)
YW = O // 4    # 192 y cols per quadrant

F32 = mybir.dt.float32
BF16 = mybir.dt.bfloat16
AF = mybir.ActivationFunctionType
ALU = mybir.AluOpType

_COMPILED = None

# bf16 const layout: WG | WF | ONES | BIAS
WG_N = NGATE * KH * 4 * 256   # 32768
WF_N = KH * 4 * YW            # 6144
NB = WG_N + WF_N + 32 + 4096  # 43040
# f32 const layout: G0 (r|hn|z|in) | H0T | IDT
NF = NGATE * 256 + 256 + 128  # 1408


def _hslice(hsb, k):
    """lhsT chunk k (h features 128k..128k+128) from packed h'^T tile."""
    c = 128 * (k % 2) + 32 * (k // 2)
    return hsb[:, c : c + 32]


def _build_nc():
    nc = bacc.Bacc("TRN2", target_bir_lowering=False, debug=False, num_devices=NCORES)

    cb = nc.declare_dram_parameter("CB", [128, NB], BF16, isOutput=False)
    cf = nc.declare_dram_parameter("CF", [128, NF], F32, isOutput=False)
    o = nc.declare_dram_parameter("O", [T, 128, YW], F32, isOutput=True)

    with tile.TileContext(nc) as tc:
        with (
            tc.tile_pool(name="wpool", bufs=1) as wpool,
            tc.tile_pool(name="state", bufs=2) as spool,
            tc.tile_pool(name="act", bufs=2) as apool,
            tc.tile_pool(name="gps", bufs=2, space="PSUM") as gpool,
            tc.tile_pool(name="tps", bufs=1, space="PSUM") as tpool,
        ):
            CB = wpool.tile([128, NB], BF16, tag="CB")
            CF = wpool.tile([128, NF], F32, tag="CF")
            nc.sync.dma_start(CB[:], cb[:])
            nc.sync.dma_start(CF[:], cf[:])
            WG = CB[:, 0:WG_N]
            WF = CB[:, WG_N : WG_N + WF_N]
            ONES = CB[0:1, WG_N + WF_N : WG_N + WF_N + 32]
            BIAS = CB[0:1, WG_N + WF_N + 32 : NB]
            G0 = CF[:, 0 : NGATE * 256]
            H0T = CF[:, NGATE * 256 : NGATE * 256 + 256]
            IDT = CF[:, NGATE * 256 + 256 : NF]

            def chain_partA(r_src, hn_src, z_src, in_src):
                """scalar: rs, zs, tanh; vector: rt, ns; PE: zs^T.
                Also allocates tpN ([0:256] for n^T, [256:448] for y)."""
                rs = apool.tile([128, 256], F32, tag="rs")
                nc.scalar.activation(rs[:], r_src, AF.Sigmoid)
                zs = apool.tile([128, 256], F32, tag="zs")
                nc.scalar.activation(zs[:], z_src, AF.Sigmoid)
                rt = apool.tile([128, 256], F32, tag="rt")
                nc.vector.tensor_tensor(rt[:], rs[:], hn_src, ALU.mult)
                ns = apool.tile([128, 256], F32, tag="ns")
                nc.vector.tensor_tensor(ns[:], rt[:], in_src, ALU.add)
                tpZ = tpool.tile([128, 256], F32, tag="tpZ")
                nc.tensor.transpose(tpZ[:, 0:128], zs[:, 0:128], IDT)
                nc.tensor.transpose(tpZ[:, 128:256], zs[:, 128:256], IDT)
                zcT = apool.tile([128, 256], F32, tag="zcT")
                nc.scalar.activation(zcT[:], tpZ[:], AF.Copy, bias=1.0, scale=-1.0)
                n = apool.tile([128, 256], F32, tag="n")
                nc.scalar.activation(n[:], ns[:], AF.Tanh)
                tpN = tpool.tile([128, 448], F32, tag="tpN")
                return n, tpZ, zcT, tpN

            def chain_partB(n, tpZ, zcT, tpN, hsb_prev):
                """PE: n^T; vector: p^T, v^T, hsb' (bf16)."""
                nc.tensor.transpose(tpN[:, 0:128], n[:, 0:128], IDT)
                nc.tensor.transpose(tpN[:, 128:256], n[:, 128:256], IDT)
                pT = apool.tile([128, 256], F32, tag="pT")
                nc.vector.tensor_tensor(pT[:], tpZ[:], hsb_prev, ALU.mult)
                vT = apool.tile([128, 256], F32, tag="vT")
                nc.vector.tensor_tensor(vT[:], tpN[:, 0:256], zcT[:], ALU.mult)
                hsb2 = spool.tile([128, 256], BF16, tag="hsb")
                nc.vector.tensor_tensor(hsb2[:], vT[:], pT[:], ALU.add)
                return hsb2

            def emit_y(hsb_t, tpN):
                for k in range(KH):
                    lhsT = _hslice(hsb_t, k)
                    for j in range(4):
                        wofs = (k * 4 + j) * YW
                        nc.tensor.matmul(
                            tpN[32 * j : 32 * j + 32, 256:448],
                            lhsT,
                            WF[:, wofs : wofs + YW],
                            start=(k == 0),
                            stop=(k == KH - 1),
                            tile_position=(0, 32 * j),
                        )

            # step 0: gates computed host-side (biases already included)
            n0, tpZ0, zcT0, tpN0 = chain_partA(
                G0[:, 0:256], G0[:, 256:512], G0[:, 512:768], G0[:, 768:1024]
            )
            hsb = chain_partB(n0, tpZ0, zcT0, tpN0, H0T)

            for t in range(T):
                last = t == T - 1
                if not last:
                    # gates for step t+1, read hsb_t
                    gA = gpool.tile([128, 512], F32, tag="gA")  # r | hn
                    gZ = gpool.tile([128, 256], F32, tag="gZ")
                    gI = gpool.tile([128, 256], F32, tag="gI")
                    # bias seeds (start=True).  gA: ONE N=512 MM per
                    # quadrant covering r|hn together (a second start in
                    # the same bank strip would clear the earlier bias).
                    for j in range(4):
                        nc.tensor.matmul(
                            gA[32 * j : 32 * j + 32, :],
                            ONES[:, 0:32],
                            BIAS[:, 512 * j : 512 * j + 512],
                            start=True, stop=False, tile_position=(0, 32 * j),
                        )
                    for gi, gt in ((2, gZ), (3, gI)):
                        for j in range(4):
                            bofs = 1024 * gi + 256 * j
                            nc.tensor.matmul(
                                gt[32 * j : 32 * j + 32, :],
                                ONES[:, 0:32],
                                BIAS[:, bofs : bofs + 256],
                                start=True, stop=False, tile_position=(0, 32 * j),
                            )
                    # region-major gate matmuls: r, hn, z, in
                    tgt = [gA[:, 0:256], gA[:, 256:512], gZ[:], gI[:]]
                    for g in range(NGATE):
                        for k in range(KH):
                            lhsT = _hslice(hsb, k)
                            for j in range(4):
                                wofs = ((g * KH + k) * 4 + j) * 256
                                nc.tensor.matmul(
                                    tgt[g][32 * j : 32 * j + 32, :],
                                    lhsT,
                                    WG[:, wofs : wofs + 256],
                                    start=False,
                                    stop=(k == KH - 1),
                                    tile_position=(0, 32 * j),
                                )
                    # chain for step t+1 (reads this cycle's PSUM); y_t
                    # goes between zs^T and n^T on the PE queue.
                    nA, tpZA, zcTA, tpNA = chain_partA(
                        gA[:, 0:256], gA[:, 256:512], gZ[:], gI[:]
                    )
                    emit_y(hsb, tpNA)
                    hsb = chain_partB(nA, tpZA, zcTA, tpNA, hsb[:])
                    ysrc = tpNA
                else:
                    tpN_last = tpool.tile([128, 448], F32, tag="tpN")
                    emit_y(hsb, tpN_last)
                    ysrc = tpN_last

                ys = apool.tile([128, YW], F32, tag="ys")
                nc.vector.tensor_copy(ys[:], ysrc[:, 256:448])
                nc.sync.dma_start(o[t], ys[:])

    nc.compile()
    return nc


def _pack_bat(M):
    """[32, 4*W] -> [128, W]: row 32j+b holds M[b, W*j : W*j+W]."""
    w = M.shape[1] // 4
    return np.ascontiguousarray(
        M.reshape(BC, 4, w).transpose(1, 0, 2).reshape(128, w)
    )


def _prep_shared(w_ih, w_hh, b_ih, b_hh, w_fc, b_fc):
    wihT = w_ih.T.astype(np.float64)  # [768, 3072]
    whhT = w_hh.T.astype(np.float64)  # [1024, 3072]
    wfcT = w_fc.T.astype(np.float64)  # [1024, 768]
    fold = wfcT @ wihT                # [1024, 3072]
    Wr = fold[:, 0:H] + whhT[:, 0:H]
    Wz = fold[:, H : 2 * H] + whhT[:, H : 2 * H]
    Win = fold[:, 2 * H : 3 * H]
    Whn = whhT[:, 2 * H : 3 * H]

    bfold = b_fc.astype(np.float64) @ wihT  # [3072]
    br = bfold[0:H] + b_ih[0:H] + b_hh[0:H]
    bz = bfold[H : 2 * H] + b_ih[H : 2 * H] + b_hh[H : 2 * H]
    bin_ = bfold[2 * H :] + b_ih[2 * H :]
    bhn = b_hh[2 * H :].astype(np.float64)

    blocks = []
    for G in (Wr, Whn, Wz, Win):  # region order r, hn, z, in
        for k in range(KH):
            for j in range(4):
                blocks.append(G[128 * k : 128 * k + 128, 256 * j : 256 * j + 256])
    WGp = np.concatenate(blocks, axis=1).astype(ml_dtypes.bfloat16)  # [128, 32768]

    yblocks = []
    for k in range(KH):
        for j in range(4):
            yblocks.append(wfcT[128 * k : 128 * k + 128, YW * j : YW * j + YW])
    WFp = np.concatenate(yblocks, axis=1).astype(ml_dtypes.bfloat16)  # [128, 6144]

    ones_col = np.zeros((128, 32), ml_dtypes.bfloat16)
    ones_col[0, :] = 1
    # bias layout: j-paired [br_j | bhn_j] (4x512) then bz (1024), bin (1024)
    bias_row = np.empty(4096, np.float64)
    for j in range(4):
        bias_row[512 * j : 512 * j + 256] = br[256 * j : 256 * j + 256]
        bias_row[512 * j + 256 : 512 * j + 512] = bhn[256 * j : 256 * j + 256]
    bias_row[2048:3072] = bz
    bias_row[3072:4096] = bin_
    bias_col = np.zeros((128, 4096), ml_dtypes.bfloat16)
    bias_col[0, :] = bias_row.astype(ml_dtypes.bfloat16)

    CBp = np.concatenate([WGp, WFp, ones_col, bias_col], axis=1)  # [128, NB]
    assert CBp.shape[1] == NB
    IDT = np.eye(128, dtype=np.float32)
    return CBp, IDT


def _build_in_maps(inputs):
    src = np.asarray(inputs["src"], np.float32)
    hidden = np.asarray(inputs["hidden"], np.float32)
    w_ih = np.asarray(inputs["w_ih"], np.float32)
    w_hh = np.asarray(inputs["w_hh"], np.float32)
    b_ih = np.asarray(inputs["b_ih"], np.float32)
    b_hh = np.asarray(inputs["b_hh"], np.float32)
    w_fc = np.asarray(inputs["w_fc"], np.float32)
    b_fc = np.asarray(inputs["b_fc"], np.float32)

    CBp, IDT = _prep_shared(w_ih, w_hh, b_ih, b_hh, w_fc, b_fc)

    # step-0 gates on host (f64): from x0=src[0], h0=hidden[0]
    x0 = src[0].astype(np.float64)
    h0 = hidden[0].astype(np.float64)
    gi0 = x0 @ w_ih.T.astype(np.float64) + b_ih.astype(np.float64)
    gh0 = h0 @ w_hh.T.astype(np.float64) + b_hh.astype(np.float64)
    g0r = gi0[:, 0:H] + gh0[:, 0:H]
    g0z = gi0[:, H : 2 * H] + gh0[:, H : 2 * H]
    g0in = gi0[:, 2 * H :]
    g0hn = gh0[:, 2 * H :]

    in_maps = []
    for c in range(NCORES):
        sl = slice(BC * c, BC * (c + 1))
        G0 = np.concatenate(
            [
                _pack_bat(g0r[sl]),
                _pack_bat(g0hn[sl]),
                _pack_bat(g0z[sl]),
                _pack_bat(g0in[sl]),
            ],
            axis=1,
        )  # [128, 1024] in region order r|hn|z|in
        HP0 = _pack_bat(h0[sl])  # [128, 256]
        H0T = np.concatenate(
            [HP0[:, 0:128].T, HP0[:, 128:256].T], axis=1
        )  # transposed-state layout
        CFp = np.concatenate([G0, H0T, IDT], axis=1).astype(np.float32)
        assert CFp.shape[1] == NF
        in_maps.append(dict(CB=CBp, CF=CFp))
    return in_maps


def kernel(src, tgt, hidden, w_ih, w_hh, b_ih, b_hh, w_fc, b_fc, **_kw):
    global _COMPILED
    b_fc = np.asarray(b_fc, np.float32)

    if _COMPILED is None:
        _COMPILED = _build_nc()
    nc = _COMPILED

    in_maps = _build_in_maps(
        dict(src=src, hidden=hidden, w_ih=w_ih, w_hh=w_hh, b_ih=b_ih,
             b_hh=b_hh, w_fc=w_fc, b_fc=b_fc)
    )

    res = run_bass_kernel_spmd(nc, in_maps, list(range(NCORES)))

    out = np.empty((T, B, O), np.float32)
    for c in range(NCORES):
        sl = slice(BC * c, BC * (c + 1))
        oc = np.asarray(res.results[c]["O"])  # [T, 128, 192]
        out[:, sl, :] = (
            oc.reshape(T, 4, BC, YW).transpose(0, 2, 1, 3).reshape(T, BC, O)
        )
    out += b_fc[None, None, :]
    return out


# revision 29
# speedup vs baseline: 1.2448x; 1.0756x over previous
"""GRU decoder Trainium2 kernel (data-parallel over batch, 8 cores).

Reference (per step t, PyTorch nn.GRU gate order r,z,n):
    gi = x @ w_ih.T + b_ih ; gh = h @ w_hh.T + b_hh
    r = sig(i_r + h_r); z = sig(i_z + h_z); n = tanh(i_n + r * h_n)
    h' = (1-z)*n + z*h ; y = h' @ w_fc.T + b_fc ; x <- y
Shapes: H=1024, O=768, B=256, T=256.  Each core handles 32 batch rows.

Structure (v6 - transposed state):
  * x_t = y_{t-1} folds into the hidden-side matmuls, so every recurrent
    matmul contracts over H=1024: regions r, hn (= h_n), z, in (= i_n).
  * The state lives ONLY as hsb = h'^T (bf16, PE lhsT layout).  The
    chain computes zs/n in normal layout, transposes zs and n (PE,
    cheap, off the critical tail), then finishes in transposed space:
        hsb' = n^T (1 - z^T) + z^T hsb
    so NOTHING follows the last vector op before the next gate matmuls.
  * Gate PSUM: pair tile [r|hn] (one bias MM of N=512 per quadrant -
    a second start=True in the same bank strip clears has_written and
    loses the earlier bias) + separate z / in banks, all double-buffered.
  * Biases seed PSUM via K=1 ones-row matmuls during the chain window.
  * Step-0 gates come from the host; b_fc is added on the host.
"""

import numpy as np
import ml_dtypes

import concourse.bass as bass
import concourse.bacc as bacc
import concourse.tile as tile
from concourse import mybir
from concourse.bass_utils import run_bass_kernel_spmd

H = 1024
O = 768
B = 256
T = 256
NCORES = 8
BC = B // NCORES  # 32 batch rows per core

KH = H // 128  # 8 contraction chunks
NGATE = 4      # regions r, hn, z, in (issue order)
YW = O // 4    # 192 y cols per quadrant

F32 = mybir.dt.float32
BF16 = mybir.dt.bfloat16
AF = mybir.ActivationFunctionType
ALU = mybir.AluOpType

_COMPILED = None

# bf16 const layout: WG | WF | ONES | BIAS
WG_N = NGATE * KH * 4 * 256   # 32768
WF_N = KH * 4 * YW            # 6144
NB = WG_N + WF_N + 32 + 4096  # 43040
# f32 const layout: G0 (r|hn|z|in) | H0T | IDT
NF = NGATE * 256 + 256 + 128  # 1408


def _hslice(hsb, k):
    """lhsT chunk k (h features 128k..128k+128) from packed h'^T tile."""
    c = 128 * (k % 2) + 32 * (k // 2)
    return hsb[:, c : c + 32]


def _build_nc():
    nc = bacc.Bacc("TRN2", target_bir_lowering=False, debug=False, num_devices=NCORES)

    cb = nc.declare_dram_parameter("CB", [128, NB], BF16, isOutput=False)
    cf = nc.declare_dram_parameter("CF", [128, NF], F32, isOutput=False)
    o = nc.declare_dram_parameter("O", [T, 128, YW], F32, isOutput=True)

    with tile.TileContext(nc) as tc:
        with (
            tc.tile_pool(name="wpool", bufs=1) as wpool,
            tc.tile_pool(name="state", bufs=2) as spool,
            tc.tile_pool(name="act", bufs=2) as apool,
            tc.tile_pool(name="gps", bufs=2, space="PSUM") as gpool,
            tc.tile_pool(name="tps", bufs=1, space="PSUM") as tpool,
        ):
            CB = wpool.tile([128, NB], BF16, tag="CB")
            CF = wpool.tile([128, NF], F32, tag="CF")
            nc.sync.dma_start(CB[:], cb[:])
            nc.sync.dma_start(CF[:], cf[:])
            WG = CB[:, 0:WG_N]
            WF = CB[:, WG_N : WG_N + WF_N]
            ONES = CB[0:1, WG_N + WF_N : WG_N + WF_N + 32]
            BIAS = CB[0:1, WG_N + WF_N + 32 : NB]
            G0 = CF[:, 0 : NGATE * 256]
            H0T = CF[:, NGATE * 256 : NGATE * 256 + 256]
            IDT = CF[:, NGATE * 256 + 256 : NF]

            def chain_partA(r_src, hn_src, z_src, in_src):
                """scalar: rs, zs, tanh; vector: rt, ns; PE: zs^T.
                Also allocates tpN ([0:256] for n^T, [256:448] for y)."""
                rs = apool.tile([128, 256], F32, tag="rs")
                nc.scalar.activation(rs[:], r_src, AF.Sigmoid)
                zs = apool.tile([128, 256], F32, tag="zs")
                nc.scalar.activation(zs[:], z_src, AF.Sigmoid)
                rt = apool.tile([128, 256], F32, tag="rt")
                nc.vector.tensor_tensor(rt[:], rs[:], hn_src, ALU.mult)
                ns = apool.tile([128, 256], F32, tag="ns")
                nc.vector.tensor_tensor(ns[:], rt[:], in_src, ALU.add)
                tpZ = tpool.tile([128, 256], F32, tag="tpZ")
                nc.tensor.transpose(tpZ[:, 0:128], zs[:, 0:128], IDT)
                nc.tensor.transpose(tpZ[:, 128:256], zs[:, 128:256], IDT)
                zcT = apool.tile([128, 256], F32, tag="zcT")
                nc.scalar.activation(zcT[:], tpZ[:], AF.Copy, bias=1.0, scale=-1.0)
                n = apool.tile([128, 256], F32, tag="n")
                nc.scalar.activation(n[:], ns[:], AF.Tanh)
                tpN = tpool.tile([128, 448], F32, tag="tpN")
                return n, tpZ, zcT, tpN

            def chain_partB(n, tpZ, zcT, tpN, hsb_prev):
                """PE: n^T; vector: p^T, v^T, hsb' (bf16)."""
                nc.tensor.transpose(tpN[:, 0:128], n[:, 0:128], IDT)
                nc.tensor.transpose(tpN[:, 128:256], n[:, 128:256], IDT)
                pT = apool.tile([128, 256], F32, tag="pT")
                nc.vector.tensor_tensor(pT[:], tpZ[:], hsb_prev, ALU.mult)
                vT = apool.tile([128, 256], F32, tag="vT")
                nc.vector.tensor_tensor(vT[:], tpN[:, 0:256], zcT[:], ALU.mult)
                hsb2 = spool.tile([128, 256], BF16, tag="hsb")
                nc.vector.tensor_tensor(hsb2[:], vT[:], pT[:], ALU.add)
                return hsb2

            def emit_y(hsb_t, tpN):
                for k in range(KH):
                    lhsT = _hslice(hsb_t, k)
                    for j in range(4):
                        wofs = (k * 4 + j) * YW
                        nc.tensor.matmul(
                            tpN[32 * j : 32 * j + 32, 256:448],
                            lhsT,
                            WF[:, wofs : wofs + YW],
                            start=(k == 0),
                            stop=(k == KH - 1),
                            tile_position=(0, 32 * j),
                        )

            # step 0: gates computed host-side (biases already included)
            n0, tpZ0, zcT0, tpN0 = chain_partA(
                G0[:, 0:256], G0[:, 256:512], G0[:, 512:768], G0[:, 768:1024]
            )
            hsb = chain_partB(n0, tpZ0, zcT0, tpN0, H0T)

            for t in range(T):
                last = t == T - 1
                if not last:
                    # gates for step t+1, read hsb_t
                    gA = gpool.tile([128, 512], F32, tag="gA")  # r | hn
                    gZ = gpool.tile([128, 256], F32, tag="gZ")
                    gI = gpool.tile([128, 256], F32, tag="gI")
                    # bias seeds (start=True).  gA: ONE N=512 MM per
                    # quadrant covering r|hn together (a second start in
                    # the same bank strip would clear the earlier bias).
                    for j in range(4):
                        nc.tensor.matmul(
                            gA[32 * j : 32 * j + 32, :],
                            ONES[:, 0:32],
                            BIAS[:, 512 * j : 512 * j + 512],
                            start=True, stop=False, tile_position=(0, 32 * j),
                        )
                    for gi, gt in ((2, gZ), (3, gI)):
                        for j in range(4):
                            bofs = 1024 * gi + 256 * j
                            nc.tensor.matmul(
                                gt[32 * j : 32 * j + 32, :],
                                ONES[:, 0:32],
                                BIAS[:, bofs : bofs + 256],
                                start=True, stop=False, tile_position=(0, 32 * j),
                            )
                    # r|hn pair as single N=512 matmuls (fewer LDW/issues)
                    for k in range(KH):
                        lhsT = _hslice(hsb, k)
                        for j in range(4):
                            wofs = (k * 4 + j) * 512
                            nc.tensor.matmul(
                                gA[32 * j : 32 * j + 32, :],
                                lhsT,
                                WG[:, wofs : wofs + 512],
                                start=False,
                                stop=(k == KH - 1),
                                tile_position=(0, 32 * j),
                            )
                    # then z, in regions (N=256, own banks)
                    for gi, gt in ((0, gZ), (1, gI)):
                        for k in range(KH):
                            lhsT = _hslice(hsb, k)
                            for j in range(4):
                                wofs = 16384 + ((gi * KH + k) * 4 + j) * 256
                                nc.tensor.matmul(
                                    gt[32 * j : 32 * j + 32, :],
                                    lhsT,
                                    WG[:, wofs : wofs + 256],
                                    start=False,
                                    stop=(k == KH - 1),
                                    tile_position=(0, 32 * j),
                                )
                    # chain for step t+1 (reads this cycle's PSUM); y_t
                    # goes between zs^T and n^T on the PE queue.
                    nA, tpZA, zcTA, tpNA = chain_partA(
                        gA[:, 0:256], gA[:, 256:512], gZ[:], gI[:]
                    )
                    emit_y(hsb, tpNA)
                    hsb = chain_partB(nA, tpZA, zcTA, tpNA, hsb[:])
                    ysrc = tpNA
                else:
                    tpN_last = tpool.tile([128, 448], F32, tag="tpN")
                    emit_y(hsb, tpN_last)
                    ysrc = tpN_last

                ys = apool.tile([128, YW], F32, tag="ys")
                nc.vector.tensor_copy(ys[:], ysrc[:, 256:448])
                nc.sync.dma_start(o[t], ys[:])

    nc.compile()
    return nc


def _pack_bat(M):
    """[32, 4*W] -> [128, W]: row 32j+b holds M[b, W*j : W*j+W]."""
    w = M.shape[1] // 4
    return np.ascontiguousarray(
        M.reshape(BC, 4, w).transpose(1, 0, 2).reshape(128, w)
    )


def _prep_shared(w_ih, w_hh, b_ih, b_hh, w_fc, b_fc):
    wihT = w_ih.T.astype(np.float64)  # [768, 3072]
    whhT = w_hh.T.astype(np.float64)  # [1024, 3072]
    wfcT = w_fc.T.astype(np.float64)  # [1024, 768]
    fold = wfcT @ wihT                # [1024, 3072]
    Wr = fold[:, 0:H] + whhT[:, 0:H]
    Wz = fold[:, H : 2 * H] + whhT[:, H : 2 * H]
    Win = fold[:, 2 * H : 3 * H]
    Whn = whhT[:, 2 * H : 3 * H]

    bfold = b_fc.astype(np.float64) @ wihT  # [3072]
    br = bfold[0:H] + b_ih[0:H] + b_hh[0:H]
    bz = bfold[H : 2 * H] + b_ih[H : 2 * H] + b_hh[H : 2 * H]
    bin_ = bfold[2 * H :] + b_ih[2 * H :]
    bhn = b_hh[2 * H :].astype(np.float64)

    blocks = []
    # r|hn interleaved per (k,j) for N=512 pair matmuls
    for k in range(KH):
        for j in range(4):
            blocks.append(Wr[128 * k : 128 * k + 128, 256 * j : 256 * j + 256])
            blocks.append(Whn[128 * k : 128 * k + 128, 256 * j : 256 * j + 256])
    # then z, in blocks (N=256)
    for G in (Wz, Win):
        for k in range(KH):
            for j in range(4):
                blocks.append(G[128 * k : 128 * k + 128, 256 * j : 256 * j + 256])
    WGp = np.concatenate(blocks, axis=1).astype(ml_dtypes.bfloat16)  # [128, 32768]

    yblocks = []
    for k in range(KH):
        for j in range(4):
            yblocks.append(wfcT[128 * k : 128 * k + 128, YW * j : YW * j + YW])
    WFp = np.concatenate(yblocks, axis=1).astype(ml_dtypes.bfloat16)  # [128, 6144]

    ones_col = np.zeros((128, 32), ml_dtypes.bfloat16)
    ones_col[0, :] = 1
    # bias layout: j-paired [br_j | bhn_j] (4x512) then bz (1024), bin (1024)
    bias_row = np.empty(4096, np.float64)
    for j in range(4):
        bias_row[512 * j : 512 * j + 256] = br[256 * j : 256 * j + 256]
        bias_row[512 * j + 256 : 512 * j + 512] = bhn[256 * j : 256 * j + 256]
    bias_row[2048:3072] = bz
    bias_row[3072:4096] = bin_
    bias_col = np.zeros((128, 4096), ml_dtypes.bfloat16)
    bias_col[0, :] = bias_row.astype(ml_dtypes.bfloat16)

    CBp = np.concatenate([WGp, WFp, ones_col, bias_col], axis=1)  # [128, NB]
    assert CBp.shape[1] == NB
    IDT = np.eye(128, dtype=np.float32)
    return CBp, IDT


def _build_in_maps(inputs):
    src = np.asarray(inputs["src"], np.float32)
    hidden = np.asarray(inputs["hidden"], np.float32)
    w_ih = np.asarray(inputs["w_ih"], np.float32)
    w_hh = np.asarray(inputs["w_hh"], np.float32)
    b_ih = np.asarray(inputs["b_ih"], np.float32)
    b_hh = np.asarray(inputs["b_hh"], np.float32)
    w_fc = np.asarray(inputs["w_fc"], np.float32)
    b_fc = np.asarray(inputs["b_fc"], np.float32)

    CBp, IDT = _prep_shared(w_ih, w_hh, b_ih, b_hh, w_fc, b_fc)

    # step-0 gates on host (f64): from x0=src[0], h0=hidden[0]
    x0 = src[0].astype(np.float64)
    h0 = hidden[0].astype(np.float64)
    gi0 = x0 @ w_ih.T.astype(np.float64) + b_ih.astype(np.float64)
    gh0 = h0 @ w_hh.T.astype(np.float64) + b_hh.astype(np.float64)
    g0r = gi0[:, 0:H] + gh0[:, 0:H]
    g0z = gi0[:, H : 2 * H] + gh0[:, H : 2 * H]
    g0in = gi0[:, 2 * H :]
    g0hn = gh0[:, 2 * H :]

    in_maps = []
    for c in range(NCORES):
        sl = slice(BC * c, BC * (c + 1))
        G0 = np.concatenate(
            [
                _pack_bat(g0r[sl]),
                _pack_bat(g0hn[sl]),
                _pack_bat(g0z[sl]),
                _pack_bat(g0in[sl]),
            ],
            axis=1,
        )  # [128, 1024] in region order r|hn|z|in
        HP0 = _pack_bat(h0[sl])  # [128, 256]
        H0T = np.concatenate(
            [HP0[:, 0:128].T, HP0[:, 128:256].T], axis=1
        )  # transposed-state layout
        CFp = np.concatenate([G0, H0T, IDT], axis=1).astype(np.float32)
        assert CFp.shape[1] == NF
        in_maps.append(dict(CB=CBp, CF=CFp))
    return in_maps


def kernel(src, tgt, hidden, w_ih, w_hh, b_ih, b_hh, w_fc, b_fc, **_kw):
    global _COMPILED
    b_fc = np.asarray(b_fc, np.float32)

    if _COMPILED is None:
        _COMPILED = _build_nc()
    nc = _COMPILED

    in_maps = _build_in_maps(
        dict(src=src, hidden=hidden, w_ih=w_ih, w_hh=w_hh, b_ih=b_ih,
             b_hh=b_hh, w_fc=w_fc, b_fc=b_fc)
    )

    res = run_bass_kernel_spmd(nc, in_maps, list(range(NCORES)))

    out = np.empty((T, B, O), np.float32)
    for c in range(NCORES):
        sl = slice(BC * c, BC * (c + 1))
        oc = np.asarray(res.results[c]["O"])  # [T, 128, 192]
        out[:, sl, :] = (
            oc.reshape(T, 4, BC, YW).transpose(0, 2, 1, 3).reshape(T, BC, O)
        )
    out += b_fc[None, None, :]
    return out
